# revision 1
# baseline (speedup 1.0000x reference)
"""Trainium2 Bass kernel for GroupedQueryAttention (sparse sliding-window + global).

Sharding: 8 cores = 2 (batch) x 4 (GQA groups). Core c handles batch c//4 and
kv-head g=c%4 together with its 4 query heads (heads 4g..4g+3). Wq/Wk/Wv are
column-sharded, Wo row-sharded; each core emits a transposed partial output
outT = (context_g @ Wo_g)^T which the host transposes and sums per batch.
"""

import sys

for _p in (
    "/opt/trn_rl_repo",
    "/root/.axon_site",
    "/root/.axon_site/_ro/pypackages",
    "/root/.axon_site/_ro/trn_rl_repo",
):
    if _p not in sys.path:
        sys.path.insert(0, _p)

from contextlib import ExitStack

import numpy as np

import concourse.bass as bass  # noqa: F401  (registers engine classes)
import concourse.tile as tile
from concourse import bacc, mybir
from concourse.bass_utils import run_bass_kernel_spmd
from concourse.masks import make_identity

B, S, DM = 2, 2048, 1024
NH, NKV, DH = 16, 4, 64
HPC = 4  # q heads per core (one full GQA group)
WINDOW, NGLOB = 256, 4
SCALE = 1.0 / np.sqrt(DH)
CAP = 15.0
EPS = 1e-8
P = 128
NT = S // P  # 16 sequence tiles
G = HPC + 1  # 4 q heads + 1 k head share L2norm/RoPE processing
F32 = mybir.dt.float32
F32R = mybir.dt.float32r
BF16 = mybir.dt.bfloat16
MULT = mybir.AluOpType.mult


def _build_kernel(ctx, tc, d):
    nc = tc.nc

    consts = ctx.enter_context(tc.tile_pool(name="consts", bufs=1))
    ident = consts.tile([P, P], F32)
    make_identity(nc, ident[:])
    ident_bf = consts.tile([P, P], BF16)
    nc.vector.tensor_copy(ident_bf[:], ident[:])

    wqkv_sb = consts.tile([P, 8, 384], BF16)
    nc.sync.dma_start(wqkv_sb[:], d["wqkv"].rearrange("(c p) n -> p c n", p=P))
    wo_sb = consts.tile([P, 2, DM], BF16)
    nc.sync.dma_start(wo_sb[:], d["wo"].rearrange("(c p) n -> p c n", p=P))
    cos_sb = consts.tile([P, NT, 32], F32)
    nc.sync.dma_start(cos_sb[:], d["cos"].rearrange("(t p) n -> p t n", p=P))
    sin_sb = consts.tile([P, NT, 32], F32)
    nc.sync.dma_start(sin_sb[:], d["sin"].rearrange("(t p) n -> p t n", p=P))
    ones1 = consts.tile([P, 1], F32)
    nc.vector.memset(ones1[:], 1.0)

    # persistent per-s-chunk tensors
    qt_pool = ctx.enter_context(tc.tile_pool(name="qt", bufs=NT))
    kt_pool = ctx.enter_context(tc.tile_pool(name="kt", bufs=NT))
    v_pool = ctx.enter_context(tc.tile_pool(name="v", bufs=NT))
    ctx_pool = ctx.enter_context(tc.tile_pool(name="ctx", bufs=8))

    xp = ctx.enter_context(tc.tile_pool(name="xp", bufs=3))
    xtp = ctx.enter_context(tc.tile_pool(name="xtp", bufs=10))
    work = ctx.enter_context(tc.tile_pool(name="work", bufs=3))
    attn = ctx.enter_context(tc.tile_pool(name="attn", bufs=3))

    ps_t = ctx.enter_context(tc.tile_pool(name="ps_t", bufs=2, space="PSUM"))
    ps_mm = ctx.enter_context(tc.tile_pool(name="ps_mm", bufs=2, space="PSUM"))
    ps_sc = ctx.enter_context(tc.tile_pool(name="ps_sc", bufs=2, space="PSUM"))
    ps_cx = ctx.enter_context(tc.tile_pool(name="ps_cx", bufs=2, space="PSUM"))

    qtiles, ktiles, vtiles = [], [], []
    ctxt = [[None] * 4, [None] * 4]
    for c in range(2):
        for sc in range(4):
            ctile = ctx_pool.tile([P, 512], BF16, name=f"ctx_{c}_{sc}", tag="ctx")
            ctxt[c][sc] = ctile

    # ---------------- Phase A: QKV projection, L2 norm, RoPE, transposes ----
    for i in range(NT):
        x_sb = xp.tile([P, DM], F32, tag="x")
        nc.sync.dma_start(x_sb[:], d["xs"][P * i : P * (i + 1), :])
        xb = xp.tile([P, DM], BF16, tag="xb")
        nc.vector.tensor_copy(xb[:], x_sb[:])

        xts = []
        for mj in range(8):
            pt = ps_t.tile([P, P], BF16, name=f"ptx_{i}_{mj}", tag="t")
            nc.tensor.transpose(pt[:], xb[:, P * mj : P * (mj + 1)], ident_bf[:])
            xt = xtp.tile([P, P], BF16, name=f"xt_{i}_{mj}", tag="xt")
            if mj % 2 == 0:
                nc.scalar.copy(xt[:], pt[:])
            else:
                nc.vector.tensor_copy(xt[:], pt[:])
            xts.append(xt)

        pq = ps_mm.tile([P, 384], F32, name=f"pqkv_{i}", tag="mm")
        for mj in range(8):
            nc.tensor.matmul(
                pq[:],
                lhsT=xts[mj][:],
                rhs=wqkv_sb[:, mj, :],
                start=(mj == 0),
                stop=(mj == 7),
            )

        # L2 normalization over d for q heads and k head (first 320 cols)
        ssq = work.tile([P, G * DH], F32, tag="ssq")
        nc.scalar.square(ssq[:], pq[:, 0 : G * DH])
        red = work.tile([P, G], F32, tag="red")
        nc.vector.tensor_reduce(
            red[:],
            ssq[:].rearrange("p (g n) -> p g n", g=G),
            axis=mybir.AxisListType.X,
            op=mybir.AluOpType.add,
        )
        nrm = work.tile([P, G], F32, tag="nrm")
        nc.scalar.sqrt(nrm[:], red[:])
        nrm2 = work.tile([P, G], F32, tag="nrm2")
        nc.vector.tensor_scalar_add(nrm2[:], nrm[:], EPS)
        rcn = work.tile([P, G], F32, tag="rcn")
        nc.vector.reciprocal(rcn[:], nrm2[:])
        qkn = work.tile([P, G * DH], F32, tag="qkn")
        nc.vector.tensor_tensor(
            qkn[:].rearrange("p (g n) -> p g n", g=G),
            pq[:, 0 : G * DH].rearrange("p (g n) -> p g n", g=G),
            rcn[:].unsqueeze(-1).broadcast_to([P, G, DH]),
            op=MULT,
        )

        # v (+ ones column for softmax sums)
        vt_i = v_pool.tile([P, 65], BF16, name=f"v_{i}", tag="v")
        nc.scalar.copy(vt_i[:, 64:65], ones1[:])
        nc.scalar.copy(vt_i[:, 0:64], pq[:, 320:384])
        vtiles.append(vt_i)

        # RoPE: rotate halves (d, d+32) with cos/sin of this s-chunk
        qv = qkn[:].rearrange("p (g n) -> p g n", g=G)
        x1, x2 = qv[:, :, 0:32], qv[:, :, 32:64]
        cb = cos_sb[:, i, :].unsqueeze(1).broadcast_to([P, G, 32])
        sbr = sin_sb[:, i, :].unsqueeze(1).broadcast_to([P, G, 32])
        rp = work.tile([P, G * DH], BF16, tag="rp")
        rv = rp[:].rearrange("p (g n) -> p g n", g=G)
        ta = work.tile([P, G * 32], F32, tag="ta")
        tb = work.tile([P, G * 32], F32, tag="tb")
        tav = ta[:].rearrange("p (g n) -> p g n", g=G)
        tbv = tb[:].rearrange("p (g n) -> p g n", g=G)
        nc.vector.tensor_tensor(tav, x1, cb, op=MULT)
        nc.vector.tensor_tensor(tbv, x2, sbr, op=MULT)
        nc.vector.tensor_sub(rv[:, :, 0:32], tav, tbv)
        nc.vector.tensor_tensor(tav, x1, sbr, op=MULT)
        nc.vector.tensor_tensor(tbv, x2, cb, op=MULT)
        nc.vector.tensor_add(rv[:, :, 32:64], tav, tbv)

        # transpose q (2x 128-col blocks = 4 heads) and k (64 cols)
        qt_i = qt_pool.tile([64, HPC * P], BF16, name=f"qt_{i}", tag="qt")
        for hp in range(2):
            ptq = ps_t.tile([P, P], BF16, name=f"ptq_{i}_{hp}", tag="t")
            nc.tensor.transpose(ptq[:], rp[:, P * hp : P * (hp + 1)], ident_bf[:])
            nc.scalar.copy(qt_i[:, (2 * hp) * P : (2 * hp) * P + P], ptq[0:64, :])
            nc.vector.tensor_copy(
                qt_i[:, (2 * hp + 1) * P : (2 * hp + 1) * P + P], ptq[64:128, :]
            )
        ptk = ps_t.tile([P, P], BF16, name=f"ptk_{i}", tag="t")
        nc.tensor.transpose(ptk[0:64, :], rp[:, 256:320], ident_bf[:])
        kt_i = kt_pool.tile([64, P], BF16, name=f"kt_{i}", tag="kt")
        nc.scalar.copy(kt_i[:], ptk[0:64, :])
        qtiles.append(qt_i)
        ktiles.append(kt_i)

    # ---------------- Phase B: banded attention --------------------------
    for t in range(NT):
        kts = list(range(max(0, t - 2), t + 1))
        mb = attn.tile([P, 3, P], BF16, tag="mb")
        nc.sync.dma_start(mb[:], d["band"][t])
        qrhs = qtiles[t][:].rearrange("p (h q) -> p h q", h=HPC)
        pcx = ps_cx.tile([65, 512], F32, name=f"pcx_{t}", tag="cx")

        for j_, kt in enumerate(kts):
            j = kt - (t - 2)
            ps = ps_sc.tile([P, 512], F32, name=f"psc_{t}_{kt}", tag="sc")
            nc.tensor.matmul(
                ps[:], lhsT=ktiles[kt][:], rhs=qrhs, start=True, stop=True
            )
            ex = attn.tile([P, 512], BF16, tag="ex")
            nc.scalar.activation(
                ex[:], ps[:], mybir.ActivationFunctionType.Exp, scale=SCALE
            )
            em = attn.tile([P, 512], BF16, tag="em")
            nc.vector.tensor_tensor(
                em[:].rearrange("p (h q) -> p h q", h=HPC),
                ex[:].rearrange("p (h q) -> p h q", h=HPC),
                mb[:, j, :].unsqueeze(1).broadcast_to([P, HPC, P]),
                op=MULT,
            )
            nc.tensor.matmul(
                pcx[:],
                lhsT=vtiles[kt][:],
                rhs=em[:],
                start=(j_ == 0),
                stop=(j_ == len(kts) - 1 and t < 3),
            )

        if t >= 3:
            gm = attn.tile([4, P], BF16, tag="gm")
            nc.sync.dma_start(gm[:], d["glob"][t])
            psg = ps_sc.tile([4, 512], F32, name=f"psg_{t}", tag="sc")
            nc.tensor.matmul(
                psg[:], lhsT=ktiles[0][:, 0:4], rhs=qrhs, start=True, stop=True
            )
            exg = attn.tile([4, 512], BF16, tag="exg")
            nc.scalar.activation(
                exg[:], psg[:], mybir.ActivationFunctionType.Exp, scale=SCALE
            )
            emg = attn.tile([4, 512], BF16, tag="emg")
            nc.vector.tensor_tensor(
                emg[:].rearrange("p (h q) -> p h q", h=HPC),
                exg[:].rearrange("p (h q) -> p h q", h=HPC),
                gm[:].unsqueeze(1).broadcast_to([4, HPC, P]),
                op=MULT,
            )
            nc.tensor.matmul(
                pcx[:],
                lhsT=vtiles[0][0:4, :],
                rhs=emg[:],
                start=False,
                stop=True,
            )

        # softmax denominators (row 64 of pcx) -> reciprocal -> broadcast
        sm = attn.tile([1, 512], F32, tag="sm")
        nc.scalar.copy(sm[:], pcx[64:65, :])
        rb = attn.tile([64, 512], F32, tag="rb")
        nc.gpsimd.partition_broadcast(rb[:], sm[:])
        rc = attn.tile([64, 512], F32, tag="rc")
        nc.vector.reciprocal(rc[:], rb[:])

        sc_, qoff = t // 4, (t % 4) * P
        for h in range(HPC):
            c, p0 = h // 2, 64 * (h % 2)
            nc.vector.tensor_tensor(
                ctxt[c][sc_][p0 : p0 + 64, qoff : qoff + P],
                pcx[0:64, h * P : (h + 1) * P],
                rc[:, h * P : (h + 1) * P],
                op=MULT,
            )

    # ---------------- Phase C: output projection (transposed) ------------
    outp = ctx.enter_context(tc.tile_pool(name="outp", bufs=4))
    for sc in range(4):
        for mo in range(8):
            po = ps_mm.tile([P, 512], F32, name=f"po_{sc}_{mo}", tag="mm")
            for c in range(2):
                nc.tensor.matmul(
                    po[:],
                    lhsT=wo_sb[:, c, P * mo : P * (mo + 1)],
                    rhs=ctxt[c][sc][:],
                    start=(c == 0),
                    stop=(c == 1),
                )
            ob = outp.tile([P, 512], F32, tag="ob")
            if mo % 2 == 0:
                nc.scalar.copy(ob[:], po[:])
            else:
                nc.vector.tensor_copy(ob[:], po[:])
            nc.sync.dma_start(
                d["outT"][P * mo : P * (mo + 1), 512 * sc : 512 * (sc + 1)], ob[:]
            )


def build_program():
    nc = bacc.Bacc("TRN2", target_bir_lowering=False, debug=False, num_devices=8)
    d = {}
    d["xs"] = nc.dram_tensor("xs", [S, DM], F32, kind="ExternalInput").ap()
    d["wqkv"] = nc.dram_tensor("wqkv", [DM, 384], BF16, kind="ExternalInput").ap()
    d["wo"] = nc.dram_tensor("wo", [256, DM], BF16, kind="ExternalInput").ap()
    d["cos"] = nc.dram_tensor("cos", [S, 32], F32, kind="ExternalInput").ap()
    d["sin"] = nc.dram_tensor("sin", [S, 32], F32, kind="ExternalInput").ap()
    d["band"] = nc.dram_tensor("band", [NT, P, 3, P], BF16, kind="ExternalInput").ap()
    d["glob"] = nc.dram_tensor("glob", [NT, 4, P], BF16, kind="ExternalInput").ap()
    d["outT"] = nc.dram_tensor("outT", [DM, S], F32, kind="ExternalOutput").ap()
    with tile.TileContext(nc) as tc, ExitStack() as ctx:
        _build_kernel(ctx, tc, d)
    nc.compile()
    return nc


def make_masks(mask_np):
    """Pack the combined (caller mask & sliding-window|global) mask into the
    banded [k, q]-oriented tiles the kernel consumes."""
    mask_np = np.asarray(mask_np).astype(bool)
    q = np.arange(S)[:, None]
    k = np.arange(S)[None, :]
    wmask = ((k <= q) & (k > q - WINDOW)) | (k < NGLOB)
    combT = (mask_np[0, 0] & wmask).T.astype(np.float32)  # [k, q]
    band = np.zeros((NT, P, 3, P), np.float32)
    glob = np.zeros((NT, 4, P), np.float32)
    for t in range(NT):
        for kt in range(max(0, t - 2), t + 1):
            j = kt - (t - 2)
            band[t, :, j, :] = combT[P * kt : P * (kt + 1), P * t : P * (t + 1)]
        if t >= 3:
            glob[t] = combT[0:NGLOB, P * t : P * (t + 1)]
    return band, glob


def make_in_maps(x, cos, sin, mask, Wq, Wk, Wv, Wo):
    import ml_dtypes

    bf = ml_dtypes.bfloat16
    x, cos, sin = (np.asarray(a, np.float32) for a in (x, cos, sin))
    Wq, Wk, Wv, Wo = (np.asarray(a, np.float32).astype(bf) for a in (Wq, Wk, Wv, Wo))
    band, glob = make_masks(mask)
    band, glob = band.astype(bf), glob.astype(bf)
    in_maps = []
    for c in range(8):
        b, g = divmod(c, 4)
        wqkv = np.concatenate(
            [
                Wq[:, 256 * g : 256 * (g + 1)],
                Wk[:, 64 * g : 64 * (g + 1)],
                Wv[:, 64 * g : 64 * (g + 1)],
            ],
            axis=1,
        )
        in_maps.append(
            {
                "xs": np.ascontiguousarray(x[b]),
                "wqkv": np.ascontiguousarray(wqkv),
                "wo": np.ascontiguousarray(Wo[256 * g : 256 * (g + 1), :]),
                "cos": np.ascontiguousarray(cos),
                "sin": np.ascontiguousarray(sin),
                "band": band,
                "glob": glob,
            }
        )
    return in_maps


_PROGRAM = None


def _get_program():
    global _PROGRAM
    if _PROGRAM is None:
        _PROGRAM = build_program()
    return _PROGRAM


def kernel(x, cos, sin, mask, Wq, Wk, Wv, Wo, _trace=False, _trace_kwargs=None):
    nc = _get_program()
    in_maps = make_in_maps(x, cos, sin, mask, Wq, Wk, Wv, Wo)
    res = run_bass_kernel_spmd(
        nc, in_maps, list(range(8)), trace=_trace, **(_trace_kwargs or {})
    )
    out = np.zeros((B, S, DM), np.float32)
    for c in range(8):
        out[c // 4] += res.results[c]["outT"].T
    if _trace:
        kernel._last_results = res
    return out

